# revision 11
# baseline (speedup 1.0000x reference)
"""Trainium2 Bass kernel for CLIPAttention (B=32, S=512, E=768, H=12, D=64).

Strategy: data-parallel over batch across 8 NeuronCores (4 batches/core).
Matmuls run in float32r (full PE rate at N>=256); every matmul operand is
produced on-chip by a compute op with float32r output dtype (required by the
BIR verifier). Per batch:
  x -> xT (PE transpose, fp32), qT/kT feature-major + v token-major
  projections, per-head causal scores (triangular; diagonal block masked with
  a constant [128,128] causal tile), fused exp+rowsum on ACT, normalize on
  DVE, PE-transpose of probs (fp32), PV matmul accumulating out^T
  feature-major, final out-projection back to token-major. Biases are folded
  into the PSUM->SBUF copies.
"""

import numpy as np
from contextlib import ExitStack

import concourse.bass as bass
import concourse.mybir as mybir
import concourse.tile as tile
from concourse import bacc
from concourse.bass_utils import run_bass_kernel_spmd
from concourse.masks import make_identity, make_causal_mask

B, S, E, H, D = 32, 512, 768, 12, 64
NCORES = 8
NB = B // NCORES          # batches per core
P = 128
KT = E // P               # 6 feature tiles
QT = S // P               # 4 token tiles
SCALE = float(D) ** -0.5  # 0.125
F32 = mybir.dt.float32
F32R = mybir.dt.float32r

AF = mybir.ActivationFunctionType
OP = mybir.AluOpType


def _build():
    nc = bacc.Bacc(trn_type="TRN2")

    hs = nc.dram_tensor("hs", [NB, S, E], F32, kind="ExternalInput")
    w_dr = {}
    b_dr = {}
    for nm in ("q", "k", "v", "o"):
        w_dr[nm] = nc.dram_tensor(f"W{nm}", [E, E], F32, kind="ExternalInput")
        b_dr[nm] = nc.dram_tensor(f"b{nm}", [E], F32, kind="ExternalInput")
    out = nc.dram_tensor("out", [NB, S, E], F32, kind="ExternalOutput")

    with ExitStack() as ctx:
        tc = ctx.enter_context(tile.TileContext(nc))

        singles = ctx.enter_context(tc.tile_pool(name="singles", bufs=1))
        xpool = ctx.enter_context(tc.tile_pool(name="xpool", bufs=2))
        xtpool = ctx.enter_context(tc.tile_pool(name="xtpool", bufs=1))
        qkvpool = ctx.enter_context(tc.tile_pool(name="qkvpool", bufs=1))
        ppool = ctx.enter_context(tc.tile_pool(name="ppool", bufs=4))
        ptpool = ctx.enter_context(tc.tile_pool(name="ptpool", bufs=2))
        dpool = ctx.enter_context(tc.tile_pool(name="dpool", bufs=4))
        otpool = ctx.enter_context(tc.tile_pool(name="otpool", bufs=1))
        opool = ctx.enter_context(tc.tile_pool(name="opool", bufs=2))

        ps_mm = ctx.enter_context(tc.tile_pool(name="ps_mm", bufs=2, space="PSUM"))
        ps_s = ctx.enter_context(tc.tile_pool(name="ps_s", bufs=2, space="PSUM"))
        ps_tp = ctx.enter_context(tc.tile_pool(name="ps_tp", bufs=2, space="PSUM"))
        ps_pv = ctx.enter_context(tc.tile_pool(name="ps_pv", bufs=2, space="PSUM"))

        # ---- constants ----
        ident = singles.tile([P, P], F32, name="ident")
        make_identity(nc, ident)
        diag_mask = singles.tile([P, P], F32, name="diag_mask")
        make_causal_mask(nc, diag_mask, mask_val=-1e9)

        # weights in SBUF as float32r (DMA'd directly; PE rounds on ingest)
        w_sb = {}
        for nm in ("q", "k", "v", "o"):
            w_sb[nm] = singles.tile([P, KT, E], F32R, name=f"w_{nm}")
            w_view = w_dr[nm].rearrange("(ko p) m -> p ko m", p=P)
            nc.sync.dma_start(out=w_sb[nm], in_=w_view.bitcast(F32R))

        # per-partition bias form for feature-major q/k
        bias_pp = {}
        for nm in ("q", "k"):
            bias_pp[nm] = singles.tile([P, KT], F32, name=f"bpp_{nm}")
            nc.sync.dma_start(
                out=bias_pp[nm], in_=b_dr[nm].rearrange("(ko p) -> p ko", p=P)
            )
        # broadcast-to-all-partitions bias form for token-major v/o
        bias_bc = {}
        for nm in ("v", "o"):
            bias_bc[nm] = singles.tile([P, E], F32, name=f"bbc_{nm}")
            src = b_dr[nm][:]
            bcast = bass.AP(tensor=src.tensor, offset=src.offset, ap=[[0, P], *src.ap])
            nc.sync.dma_start(out=bias_bc[nm], in_=bcast)

        NSPLIT = 384  # N-tile for the two token-major projections (768 = 2x384)

        for b in range(NB):
            # ---- stage A: load x, transpose to feature-major xT [768, 512] ----
            xt = xtpool.tile([P, KT, S], F32R, name=f"xt_{b}", tag="xt")
            for i in range(QT):
                x_t = xpool.tile([P, E], F32, name=f"x_{b}_{i}", tag="x")
                nc.sync.dma_start(out=x_t, in_=hs[b, i * P:(i + 1) * P, :])
                for j in range(KT):
                    tp = ps_tp.tile([P, P], F32, name=f"tpx_{b}_{i}_{j}", tag="tp")
                    nc.tensor.transpose(tp, x_t[:, j * P:(j + 1) * P], ident)
                    nc.scalar.copy(out=xt[:, j, i * P:(i + 1) * P], in_=tp)

            # ---- stage B: qT, kT feature-major [768, 512] ----
            qkv = {}
            for nm in ("q", "k"):
                dst = qkvpool.tile([P, KT, S], F32R, name=f"{nm}T_{b}", tag=f"{nm}T")
                qkv[nm] = dst
                for m in range(KT):
                    ps = ps_mm.tile([P, S], F32, name=f"ps{nm}_{b}_{m}", tag="mm")
                    for kk in range(KT):
                        nc.tensor.matmul(
                            ps,
                            lhsT=w_sb[nm][:, kk, m * P:(m + 1) * P],
                            rhs=xt[:, kk, :],
                            start=(kk == 0),
                            stop=(kk == KT - 1),
                        )
                    nc.scalar.activation(
                        out=dst[:, m, :],
                        in_=ps,
                        func=AF.Identity,
                        bias=bias_pp[nm][:, m:m + 1],
                        scale=1.0,
                    )

            # ---- stage C: v token-major [512, 768] ----
            v_t = qkvpool.tile([P, QT, E], F32R, name=f"v_{b}", tag="v")
            for i in range(QT):
                for n in range(E // NSPLIT):
                    ps = ps_mm.tile([P, S], F32, name=f"psv_{b}_{i}_{n}", tag="mm")
                    for kk in range(KT):
                        nc.tensor.matmul(
                            ps[:, :NSPLIT],
                            lhsT=xt[:, kk, i * P:(i + 1) * P],
                            rhs=w_sb["v"][:, kk, n * NSPLIT:(n + 1) * NSPLIT],
                            start=(kk == 0),
                            stop=(kk == KT - 1),
                        )
                    nc.vector.tensor_tensor(
                        out=v_t[:, i, n * NSPLIT:(n + 1) * NSPLIT],
                        in0=ps[:, :NSPLIT],
                        in1=bias_bc["v"][:, n * NSPLIT:(n + 1) * NSPLIT],
                        op=OP.add,
                    )

            # ---- stage D: attention heads ----
            outT = otpool.tile([P, KT, S], F32R, name=f"outT_{b}", tag="outT")
            po = None
            for h in range(H):
                g, rr = h // 2, h % 2
                pT = ptpool.tile([P, QT, S], F32R, name=f"pT_{b}_{h}", tag="pT")
                for j in range(1, QT):
                    nc.gpsimd.memset(pT[:, j, 0:j * P].bitcast(F32), 0.0)
                den = dpool.tile([P, QT], F32, name=f"den_{b}_{h}", tag="den")
                rden = dpool.tile([P, QT], F32, name=f"rden_{b}_{h}", tag="rden")

                qh = qkv["q"][rr * D:(rr + 1) * D, g, :]
                kh = qkv["k"][rr * D:(rr + 1) * D, g, :]

                for i in range(QT):
                    n_i = (i + 1) * P
                    ps = ps_s.tile([P, S], F32, name=f"pss_{b}_{h}_{i}", tag="s")
                    nc.tensor.matmul(
                        ps[:, :n_i],
                        lhsT=qh[:, i * P:(i + 1) * P],
                        rhs=kh[:, :n_i],
                        start=True,
                        stop=True,
                    )
                    # causal mask on the diagonal block
                    nc.vector.tensor_tensor(
                        out=ps[:, i * P:n_i],
                        in0=ps[:, i * P:n_i],
                        in1=diag_mask,
                        op=OP.add,
                    )
                    p_t = ppool.tile([P, S], F32, name=f"p_{b}_{h}_{i}", tag="p")
                    nc.scalar.activation(
                        out=p_t[:, :n_i],
                        in_=ps[:, :n_i],
                        func=AF.Exp,
                        scale=SCALE,
                        accum_out=den[:, i:i + 1],
                    )
                    nc.vector.reciprocal(rden[:, i:i + 1], den[:, i:i + 1])
                    nc.vector.tensor_scalar_mul(
                        p_t[:, :n_i], p_t[:, :n_i], rden[:, i:i + 1]
                    )
                    for j in range(i + 1):
                        tp = ps_tp.tile(
                            [P, P], F32, name=f"tpp_{b}_{h}_{i}_{j}", tag="tp"
                        )
                        nc.tensor.transpose(tp, p_t[:, j * P:(j + 1) * P], ident)
                        cp = (
                            nc.scalar.copy
                            if (i + j) % 2 == 0
                            else nc.vector.tensor_copy
                        )
                        cp(out=pT[:, j, i * P:n_i], in_=tp)

                # PV: accumulate out^T for this head (f32r matmuls must write
                # PSUM at partition base 0; odd heads reach partitions 64:128
                # of outT via DMA, which can shift partitions)
                po = ps_pv.tile([D, S], F32, name=f"po_{b}_{h}", tag="pv")
                for j in range(QT):
                    nc.tensor.matmul(
                        po,
                        lhsT=v_t[:, j, h * D:(h + 1) * D],
                        rhs=pT[:, j, :],
                        start=(j == 0),
                        stop=(j == QT - 1),
                    )
                if rr == 0:
                    nc.vector.tensor_copy(out=outT[0:D, g, :], in_=po)
                else:
                    potmp = dpool.tile([D, S], F32, name=f"pot_{b}_{h}", tag="potmp")
                    nc.scalar.copy(out=potmp, in_=po)
                    nc.sync.dma_start(
                        out=outT[D:2 * D, g, :].bitcast(F32), in_=potmp
                    )

            # ---- stage E: final projection, token-major out ----
            for i in range(QT):
                o_t = opool.tile([P, E], F32, name=f"o_{b}_{i}", tag="o")
                for n in range(E // NSPLIT):
                    ps = ps_mm.tile([P, S], F32, name=f"pso_{b}_{i}_{n}", tag="mm")
                    for kk in range(KT):
                        nc.tensor.matmul(
                            ps[:, :NSPLIT],
                            lhsT=outT[:, kk, i * P:(i + 1) * P],
                            rhs=w_sb["o"][:, kk, n * NSPLIT:(n + 1) * NSPLIT],
                            start=(kk == 0),
                            stop=(kk == KT - 1),
                        )
                    nc.vector.tensor_tensor(
                        out=o_t[:, n * NSPLIT:(n + 1) * NSPLIT],
                        in0=ps[:, :NSPLIT],
                        in1=bias_bc["o"][:, n * NSPLIT:(n + 1) * NSPLIT],
                        op=OP.add,
                    )
                nc.sync.dma_start(out=out[b, i * P:(i + 1) * P, :], in_=o_t)

    nc.compile()
    return nc


_NC_CACHE = None


def _get_nc():
    global _NC_CACHE
    if _NC_CACHE is None:
        _NC_CACHE = _build()
    return _NC_CACHE


def run(inputs, trace=False):
    hs = np.ascontiguousarray(np.asarray(inputs["hidden_states"], dtype=np.float32))
    assert hs.shape == (B, S, E)
    wb = {}
    for nm in ("q", "k", "v", "o"):
        wb[f"W{nm}"] = np.ascontiguousarray(
            np.asarray(inputs[f"W{nm}"], dtype=np.float32)
        )
        wb[f"b{nm}"] = np.ascontiguousarray(
            np.asarray(inputs[f"b{nm}"], dtype=np.float32)
        )

    nc = _get_nc()
    in_maps = []
    for c in range(NCORES):
        m = {"hs": hs[c * NB:(c + 1) * NB]}
        m.update(wb)
        in_maps.append(m)
    res = run_bass_kernel_spmd(
        nc, in_maps, core_ids=list(range(NCORES)), trace=trace
    )
    outp = np.concatenate([r_["out"] for r_ in res.results], axis=0)
    return outp, res


def kernel(**inputs) -> np.ndarray:
    outp, _ = run(inputs, trace=False)
    return outp
